# revision 10
# baseline (speedup 1.0000x reference)
"""Trainium2 Bass kernel for the DMP (dynamic movement primitives) rollout.

Math: the reference rollout is, per dimension d, a linear 2-state recurrence
    s_t = A s_{t-1} + B u_t,   s = [y; dy],  s_0 = [y0; 0]
with constant A (2x2), B = [dt^2; dt], and forcing
    u_t[d] = ALPHA_Y*BETA_Y*g[d] + sum_j phi_t[j] * weights[d,j]*(g[d]-y0[d])
where phi_t[j] = WEIGHT_SCALE * psi_t[j] * x_t / sum(psi_t) depends only on
constants (x_t = decay^t is input-independent).  By superposition the whole
trajectory factors through an input-independent basis:
    y_t[d], dy_t[d] = sum_m BB[t, comp, m] * coeff[m, d]       (m = 0..26)
with channels m = 0..24 the 25 basis-forced responses (coeff w[:,j]*(g-y0)),
m = 25 the homogeneous response (coeff y0), m = 26 the step response with
ALPHA_Y*BETA_Y folded in (coeff g).

Per core (time rows sharded across 8 cores, no cross-core comm):
  - one fp16 input blob [27, 3584] holds the transposed basis slice
    (cols 0:2560) and the pre-coefficient block {w.T; y0; g} (cols 2560:3584),
  - coeff rows 0..24 = w.T * (g - y0): the (g - y0) partition-broadcast is a
    ones[1,25] tensor-engine matmul (no DRAM roundtrip),
  - the y/dy output blocks are [27,128]x[27,512] fp16 matmuls; PSUM->SBUF
    copies alternate between DVE and ACT so both hide under the DMA,
  - the y0-replica block is one DRAM->DRAM partition-broadcast DMA, issued
    right after the blob load so it fills the DMA pipe while the matmul
    chain ramps (nothing compute-dependent may queue ahead of it),
  - the output tensor is fp16 (the harness gate is rel_err < 2e-2; fp16
    rounding costs ~3e-4) which halves the mandatory HBM write traffic --
    the kernel is output-write bound: the DMA device runs gap-free from
    first input load to last output store (~21.9us busy of ~25.5us total;
    the rest is fixed head latency and the final semaphore/drain tail).
"""

import numpy as np

DIM = 1024
NB = 25
ALPHA_X = 1.0
DT = 0.001
MAX_TIME = 10.0
TAU = 1.0
ALPHA_Y = 25.0
BETA_Y = 6.25
WEIGHT_SCALE = 1000.0
T = int(MAX_TIME / DT) + 1        # 10001

NCORES = 8
RPC = 1251                        # t-rows per core; 8*1251 = 10008 >= T
R2 = RPC * 2                      # 2502 matmul rows per core (y and dy)
R2PAD = 2560                      # 20 tiles of 128
NMT = R2PAD // 128                # 20
M = 2 + NB                        # 27 basis channels
CB = R2PAD + DIM                  # blob cols: basis | {w.T; y0; g}

_cache = {}


def _basis_slices():
    """Per-core transposed basis slices: list of [M, R2PAD] float32 arrays."""
    if "bbT" in _cache:
        return _cache["bbT"]
    f32 = np.float32
    # phi replicated in fp32 with the reference op order
    c = np.exp(-ALPHA_X * np.linspace(0.0, MAX_TIME, NB, dtype=f32)).astype(f32)
    h = (NB / c).astype(f32)
    decay = f32(1.0 - ALPHA_X * TAU * DT)
    x = f32(1.0)
    phi = np.zeros((T - 1, NB), dtype=np.float64)
    for t in range(T - 1):
        x = f32(x * decay)
        d = (x - c).astype(f32)
        arg = (h * (d * d).astype(f32)).astype(f32)
        psi = np.exp(-arg).astype(f32)
        s = f32(psi.sum(dtype=f32))
        phi[t] = (psi.astype(np.float64) * float(x) * WEIGHT_SCALE) / float(s)

    dt = TAU * DT
    a, b = ALPHA_Y, BETA_Y
    A = np.array([[1 - dt * dt * a * b, dt * (1 - dt * a)],
                  [-dt * a * b, 1 - dt * a]], dtype=np.float64)
    B = np.array([dt * dt, dt], dtype=np.float64)
    # internal channel order: 0 homogeneous (E), 1 step (S), 2.. forced (C)
    Z = np.zeros((2, M), dtype=np.float64)
    Z[0, 0] = 1.0
    # output channel order (must match device rhs rows):
    #   m = 0..24 -> C_j (coeff w.T*(g-y0)); m = 25 -> E (coeff y0);
    #   m = 26 -> ALPHA_Y*BETA_Y*S (coeff g, scale folded into the basis)
    BB = np.zeros((T, 2, M), dtype=np.float64)
    BB[0, 0, 25] = 1.0                 # y_0 = y0 (dy_0 row stays zero)
    u = np.zeros(M)
    u[1] = 1.0
    for t in range(1, T):
        u[2:] = phi[t - 1]
        Z = A @ Z + np.outer(B, u)
        for comp in (0, 1):
            BB[t, comp, :25] = Z[comp, 2:]
            BB[t, comp, 25] = Z[comp, 0]
            BB[t, comp, 26] = (a * b) * Z[comp, 1]

    flat = np.zeros((NCORES * R2, M), dtype=f32)
    flat[: T * 2] = BB.reshape(T * 2, M).astype(f32)
    slices = []
    for i in range(NCORES):
        bbT = np.zeros((M, R2PAD), dtype=f32)
        bbT[:, :R2] = flat[i * R2:(i + 1) * R2].T
        slices.append(np.ascontiguousarray(bbT))
    _cache["bbT"] = slices
    return slices


def _program():
    """Build (once) the Bass/Tile program shared by all 8 cores."""
    if "nc" in _cache:
        return _cache["nc"]
    import concourse.mybir as mybir
    import concourse.tile as tile
    from concourse import bacc

    f16 = mybir.dt.float16
    f32 = mybir.dt.float32
    nc = bacc.Bacc("TRN2", target_bir_lowering=False, debug=False,
                   enable_asserts=False, num_devices=NCORES)
    blob_h = nc.dram_tensor("blob", [M, CB], f16, kind="ExternalInput")
    yg_h = nc.dram_tensor("yg", [1, 2 * DIM], f16, kind="ExternalInput")
    out_h = nc.dram_tensor("out", [RPC, 3, DIM], f16, kind="ExternalOutput")

    with tile.TileContext(nc) as tc:
        with (
            tc.tile_pool(name="const", bufs=1) as const,
            tc.tile_pool(name="psB", bufs=1, space="PSUM") as psB,
            tc.tile_pool(name="psMM", bufs=3, space="PSUM") as psMM,
            tc.tile_pool(name="outp", bufs=6) as outp,
        ):
            outv = out_h.ap()
            blobv = blob_h.ap()

            # main input load first: everything matmul-side hangs off it, and
            # its transfer must clear the (FIFO) DMA device before the bulk
            # y0-replica broadcast below.
            blob_s = const.tile([M, CB], f16)
            nc.sync.dma_start(blob_s[:], blobv[:])

            # y0-replica output block: a single DRAM->DRAM partition-broadcast
            # write.  No compute dependency -- fills the DMA pipe while the
            # matmul chain ramps.
            nc.sync.dma_start(outv[0:RPC, 0, :],
                              yg_h.ap()[0:1, 0:DIM].broadcast_to([RPC, DIM]))

            # small load on the gpsimd (SWDGE) queue: keeps the HWDGE ring
            # free for the bulk writes above
            yg_s = const.tile([1, 2 * DIM], f16)
            nc.gpsimd.dma_start(yg_s[:], yg_h.ap()[:])

            # g - y0 broadcast to 25 partitions via a ones[1,25] matmul.
            # Everything is split into 512-col halves so the first main
            # matmuls start as soon as half 0 of the rhs is ready, and the
            # tensor engine stays continuously busy (p-state ramp).
            ones25 = const.tile([1, 32], f16)
            nc.vector.memset(ones25[:], 1.0)
            gmy0 = const.tile([1, DIM], f16)
            gm25 = psB.tile([NB, DIM], f32)
            for nh in range(2):
                ns = slice(nh * 512, (nh + 1) * 512)
                nc.vector.tensor_sub(gmy0[:, ns], yg_s[:, DIM + nh * 512:
                                                       DIM + (nh + 1) * 512],
                                     yg_s[:, nh * 512:(nh + 1) * 512])
                for q in range(4):
                    qs = slice(nh * 512 + q * 128, nh * 512 + (q + 1) * 128)
                    nc.tensor.matmul(gm25[:, qs], ones25[:, 0:NB], gmy0[:, qs],
                                     start=True, stop=True)
                # rhs rows 0..24: w.T * (g - y0), multiplied IN-PLACE in the
                # blob (rows 25/26 -- y0, g -- are already there from the blob
                # load, so no separate row DMA that could queue behind the
                # bulk y0 write)
                bs = slice(R2PAD + nh * 512, R2PAD + (nh + 1) * 512)
                nc.vector.tensor_mul(blob_s[0:NB, bs], blob_s[0:NB, bs],
                                     gm25[:, ns])

            # main matmul: [2502, 27] @ [27, 1024], tiled [128, 512]; each
            # 128-row tile covers 64 t-rows x {y, dy}
            for mt in range(NMT):
                ms = slice(mt * 128, (mt + 1) * 128)
                ps = psMM.tile([128, DIM], f32)
                for nh in range(2):
                    ns = slice(R2PAD + nh * 512, R2PAD + (nh + 1) * 512)
                    nc.tensor.matmul(ps[:, nh * 512:(nh + 1) * 512],
                                     blob_s[0:M, ms], blob_s[0:M, ns],
                                     start=True, stop=True)
                ob = outp.tile([128, DIM], f16)
                # alternate PSUM->SBUF copies between ACT and DVE (ACT first:
                # DVE is still finishing the rhs multiply when ob0 is due)
                if mt % 2 == 0:
                    nc.scalar.copy(ob[:], ps[:])
                else:
                    nc.vector.tensor_copy(ob[:], ps[:])
                t0 = mt * 64
                tv = min(64, RPC - t0)
                nc.sync.dma_start(outv[t0:t0 + tv, 1:3, :], ob[:2 * tv, :])

    nc.compile()   # bacc passes: wait legalization (1-wait HW cap), regalloc
    _cache["nc"] = nc
    return nc


def _run(in_maps, **kwargs):
    from concourse.bass_utils import run_bass_kernel_spmd
    return run_bass_kernel_spmd(_program(), in_maps, core_ids=list(range(NCORES)),
                                **kwargs)


def _in_maps(y0, g, weights):
    f16 = np.float16
    y0f = np.asarray(y0, np.float32).reshape(DIM)
    gf = np.asarray(g, np.float32).reshape(DIM)
    wf = np.asarray(weights, np.float32).reshape(DIM, NB)
    yg = np.concatenate([y0f, gf]).astype(f16).reshape(1, 2 * DIM)
    pre = np.zeros((M, DIM), dtype=f16)
    pre[0:NB] = wf.T.astype(f16)
    pre[NB] = y0f.astype(f16)
    pre[NB + 1] = gf.astype(f16)
    maps = []
    for bbT in _basis_slices():
        blob = np.zeros((M, CB), dtype=f16)
        blob[:, :R2PAD] = bbT.astype(f16)
        blob[:, R2PAD:] = pre
        maps.append({"blob": blob, "yg": yg})
    return maps


def kernel(y0, g, weights, **_kwargs):
    res = _run(_in_maps(y0, g, weights))
    outs = [r["out"].reshape(RPC, 3 * DIM) for r in res.results]
    full = np.concatenate(outs, axis=0)[:T]
    return np.ascontiguousarray(full.astype(np.float32))
